# revision 7
# baseline (speedup 1.0000x reference)
"""CircularMaxPool2d (disk stencil, radius 5, reflect padding) on 8 TRN2 NeuronCores.

Input x: [8, 1, 2048, 2048] f32. Data-parallel: core c processes batch c.

Disk decomposition (radius 5; row widths by |dy|: 11,9,9,9,7,1):

  out[r,c] = max( h11[r,c], max_{|d|<=3} h9[r+d,c], h7[r-4,c], h7[r+4,c],
                  x[r-5,c], x[r+5,c] )

where hk = horizontal k-wide centered running max of x. The horizontal
maxes nest: s1 (2w) -> s2 (4w) -> h7 = s2 over +3 -> h9 = h7 over +-1 ->
h11 = h9 over +-1 (5 ops for all three widths). Vertical: 2-level ladder
t1/t2 over h9 plus direct taps. 13 DVE ops per tile, ~13.7 elem/output.

All compute is fp16 (output tolerance 2e-2, fp16 rounding ~5e-4): DVE runs
16-bit packed tensor_tensor at 2 elem/cycle/lane (2x_1p mode). Only DVE can
run tensor_tensor max on TRN2 (the Pool/GPSIMD engine ISA rejects it), so
the kernel is a single DVE stream with DMA double-buffered underneath.

Layout: each partition owns a (column-chunk, row-group) pair: G=128
consecutive rows x WB=64 columns. The input is packed on the host into a
blocked [superband, 128, G+10, WB+10] fp16 tensor with vertical halo rows
and reflect padding baked in, so every HBM load is fully contiguous and
every vertical shift is a free-dim offset. No on-device halo exchange.
Output is written blocked fp16 and unscrambled on the host.
"""

import sys

sys.path.insert(0, "/opt/trn_rl_repo")

import numpy as np

H = 2048
W = 2048
RAD = 5
P = 128
G = 128  # rows per partition group
NG = H // G  # 16 row groups
NCHUNK = P // NG  # 8 column chunks per superband
WB = 64  # cols per chunk
WH = WB + 12  # 76: left halo 6, right halo 5 + 1 pad (even strides)
NSB = W // (WB * NCHUNK)  # 4 superbands
XR = G + 2 * RAD  # 138 rows in x tile
N_CORES = 8

_CACHE = {}


def _build():
    import concourse.bacc as bacc
    import concourse.tile as tile
    import concourse.mybir as mybir

    f16 = mybir.dt.float16
    MAX = mybir.AluOpType.max

    nc = bacc.Bacc("TRN2", target_bir_lowering=False, debug=False, num_devices=N_CORES)
    xin = nc.dram_tensor("xin", [NSB, P, XR, WH], f16, kind="ExternalInput").ap()
    yout = nc.dram_tensor("yout", [NSB, P, G, WB], f16, kind="ExternalOutput").ap()

    with tile.TileContext(nc) as tc:
        with (
            tc.tile_pool(name="xx", bufs=2) as p_xx,
            tc.tile_pool(name="pA", bufs=1) as p_a,
            tc.tile_pool(name="pB", bufs=1) as p_b,
            tc.tile_pool(name="pC", bufs=1) as p_c,
            tc.tile_pool(name="acc", bufs=2) as p_acc,
        ):
            for b in range(NSB):
                # ---- load packed band (halos pre-baked); rows: slot i = row+5
                xx = p_xx.tile([P, XR, WH], f16, tag="xx")
                # s1[i,j] = max over x cols {j-5, j-4} at row i-4 (slot i+1 in xx)
                s1 = p_a.tile([P, G + 8, 75], f16, tag="A")
                if b == 0:
                    # split the cold-start load 4 ways so the ladder starts
                    # after the first quarter lands
                    cuts = [0, 35, 69, 104, XR]
                    for k in range(4):
                        nc.sync.dma_start(
                            xx[:, cuts[k] : cuts[k + 1], :],
                            xin[b][:, cuts[k] : cuts[k + 1], :],
                        )
                    # s1 chunk k needs xx rows [cuts[k-1]+1, cuts[k]+1) ... i.e.
                    # chunk k of s1 rows [cuts[k]-1, cuts[k+1]-1) reads xx rows
                    # [cuts[k], cuts[k+1]) plus one row from the next chunk; use
                    # rows [max(cuts[k]-1,0), min(cuts[k+1]-1, G+8)) so chunk k
                    # only reads xx rows < cuts[k+1]+... keep it simple: chunk k
                    # covers s1 rows [lo, hi) with lo=cuts[k]-1 (clamped), which
                    # reads xx rows [lo+1, hi+1) <= cuts[k+1].
                    for k in range(4):
                        lo = max(cuts[k] - 1, 0)
                        hi = min(cuts[k + 1] - 1, G + 8)
                        nc.vector.tensor_tensor(
                            s1[:, lo:hi, :],
                            xx[:, lo + 1 : hi + 1, 0:75],
                            xx[:, lo + 1 : hi + 1, 1:76],
                            op=MAX,
                        )
                else:
                    nc.sync.dma_start(xx[:, :, :], xin[b])
                    nc.vector.tensor_tensor(
                        s1[:, :, :],
                        xx[:, 1 : G + 9, 0:75],
                        xx[:, 1 : G + 9, 1:76],
                        op=MAX,
                    )
                # s2[i,j] = 4-wide, x cols j-5..j-2, rows -4..G+3 (slot i = row+4)
                s2 = p_b.tile([P, G + 8, 73], f16, tag="B")
                nc.vector.tensor_tensor(
                    s2[:, :, :], s1[:, :, 0:73], s1[:, :, 2:75], op=MAX
                )
                # h7[i,j] = 7-wide centered at col j-2, rows -4..G+3
                h7 = p_c.tile([P, G + 8, 70], f16, tag="C")
                nc.vector.tensor_tensor(
                    h7[:, :, :], s2[:, :, 0:70], s2[:, :, 3:73], op=MAX
                )
                # h9[k,j] = 9-wide centered at col j-1, rows -3..G+2 (slot k = row+3)
                h9 = p_a.tile([P, G + 6, 68], f16, tag="A")
                nc.vector.tensor_tensor(
                    h9[:, :, :], h7[:, 1 : G + 7, 0:68], h7[:, 1 : G + 7, 2:70], op=MAX
                )
                # h11 (11-wide centered, dy=0) straight into the accumulator
                acc = p_acc.tile([P, G, WB], f16, tag="acc")
                nc.vector.tensor_tensor(
                    acc[:, :, :],
                    h9[:, 3 : G + 3, 1 : 1 + WB],
                    h9[:, 3 : G + 3, 3 : 3 + WB],
                    op=MAX,
                )
                # t1[m,c] = max(h9[m], h9[m+1]) -> rows m-3..m-2, 9-wide center c
                # t1 emitted with transposed (column-major) APs: measures
                # whether fully-strided streams run at full DVE rate.
                t1 = p_b.tile([P, G + 5, WB], f16, tag="B")
                nc.vector.tensor_tensor(
                    t1[:, :, :].transpose([0, 2, 1]),
                    h9[:, 0 : G + 5, 2 : 2 + WB].transpose([0, 2, 1]),
                    h9[:, 1 : G + 6, 2 : 2 + WB].transpose([0, 2, 1]),
                    op=MAX,
                )
                # t2[m,c] = max(t1[m], t1[m+2]) -> rows m-3..m
                t2 = p_a.tile([P, G + 3, WB], f16, tag="A")
                nc.vector.tensor_tensor(
                    t2[:, :, :], t1[:, 0 : G + 3, :], t1[:, 2 : G + 5, :], op=MAX
                )
                # acc = max(acc, t2[r], t2[r+3])       -> h9 rows r-3..r+3
                #       max(acc, h7[r-4], h7[r+4])     (h7 slot = r-+4+4, col j=c+2)
                #       max(acc, x[r-5], x[r+5])       (xx slot = r-+5+5, col c+5)
                # On the last superband, run the taps in two row-halves and
                # store the first half early to hide the output DMA tail.
                halves = [(0, G)] if b < NSB - 1 else [(0, G // 2), (G // 2, G)]
                for h0, h1 in halves:
                    a = acc[:, h0:h1, :]
                    nc.vector.tensor_tensor(a, a, t2[:, h0:h1, :], op=MAX)
                    nc.vector.tensor_tensor(a, a, t2[:, 3 + h0 : 3 + h1, :], op=MAX)
                    nc.vector.tensor_tensor(a, a, h7[:, h0:h1, 3 : 3 + WB], op=MAX)
                    nc.vector.tensor_tensor(
                        a, a, h7[:, 8 + h0 : 8 + h1, 3 : 3 + WB], op=MAX
                    )
                    nc.vector.tensor_tensor(a, a, xx[:, h0:h1, 6 : 6 + WB], op=MAX)
                    nc.vector.tensor_tensor(
                        a, a, xx[:, 10 + h0 : 10 + h1, 6 : 6 + WB], op=MAX
                    )
                    nc.scalar.dma_start(
                        yout[b][:, h0:h1, :], acc[:, h0:h1, :]
                    )

    nc.compile()
    return nc


def _get_nc():
    if "nc" not in _CACHE:
        _CACHE["nc"] = _build()
    return _CACHE["nc"]


def _pack_input(img):
    """[2048, 2048] f32 -> [NSB, 128, XR, WH] fp16 with reflect pad + halos.

    Partition p of superband s holds rows [G*g-5, G*g+G+5) and cols
    [(NCHUNK*s + c)*WB - 5, ... + WB + 5) of the original image, where
    c = p // NG, g = p % NG (indices in reflect-padded coordinates).
    """
    xpad = np.pad(img, ((RAD, RAD), (6, 6)), mode="reflect")  # [2058, 2060]
    wv = np.lib.stride_tricks.sliding_window_view(xpad, XR, axis=0)
    wv = wv[::G].transpose(0, 2, 1)  # [NG, XR, 2058]
    out = np.empty((NSB, P, XR, WH), dtype=np.float16)
    for s in range(NSB):
        for c in range(NCHUNK):
            j0 = (NCHUNK * s + c) * WB
            out[s, c * NG : (c + 1) * NG] = wv[:, :, j0 : j0 + WH]
    return out


def _unpack_output(yblk):
    """[NSB, 128, G, WB] fp16 -> [2048, 2048] f32."""
    y = np.empty((H, W), dtype=np.float32)
    for s in range(NSB):
        for c in range(NCHUNK):
            j0 = (NCHUNK * s + c) * WB
            blk = yblk[s, c * NG : (c + 1) * NG]  # [NG, G, WB]
            y[:, j0 : j0 + WB] = blk.reshape(H, WB).astype(np.float32)
    return y


def kernel(x, radius):
    from concourse.bass_utils import run_bass_kernel_spmd

    assert int(radius) == RAD
    x = np.asarray(x, dtype=np.float32)
    B, C = x.shape[0], x.shape[1]
    imgs = x.reshape(B * C, H, W)
    assert imgs.shape[0] == N_CORES

    imgs = np.where(np.isnan(imgs), np.float32(-99.0), imgs)

    nc = _get_nc()
    in_maps = [{"xin": _pack_input(imgs[c])} for c in range(N_CORES)]
    res = run_bass_kernel_spmd(nc, in_maps, core_ids=list(range(N_CORES)), trace=False)
    out = np.empty((N_CORES, H, W), dtype=np.float32)
    for c in range(N_CORES):
        out[c] = _unpack_output(res.results[c]["yout"])
    out = out.reshape(B, C, H, W)
    out = np.where(out == np.float32(-99.0), np.float32(np.nan), out)
    return out.astype(np.float32)


# revision 8
# speedup vs baseline: 1.4901x; 1.4901x over previous
"""CircularMaxPool2d (disk stencil, radius 5, reflect padding) on 8 TRN2 NeuronCores.

Input x: [8, 1, 2048, 2048] f32. Data-parallel: core c processes batch c.

Disk decomposition (radius 5; row widths by |dy|: 11,9,9,9,7,1):

  out[r,c] = max( h11[r,c], max_{|d|<=3} h9[r+d,c], h7[r-4,c], h7[r+4,c],
                  x[r-5,c], x[r+5,c] )

where hk = horizontal k-wide centered running max of x. The horizontal
maxes nest: s1 (2w) -> s2 (4w) -> h7 = s2 over +3 -> h9 = h7 over +-1 ->
h11 = h9 over +-1 (5 ops for all three widths). Vertical: 2-level ladder
t1/t2 over h9 plus direct taps. 13 DVE ops per tile, ~13.7 elem/output.

All compute is fp16 (output tolerance 2e-2, fp16 rounding ~5e-4): DVE runs
16-bit packed tensor_tensor at 2 elem/cycle/lane (2x_1p mode). Only DVE can
run tensor_tensor max on TRN2 (the Pool/GPSIMD engine ISA rejects it), so
the kernel is a single DVE stream with DMA double-buffered underneath.

Layout: each partition owns a (column-chunk, row-group) pair: G=128
consecutive rows x WB=64 columns. The input is packed on the host into a
blocked [superband, 128, G+10, WB+10] fp16 tensor with vertical halo rows
and reflect padding baked in, so every HBM load is fully contiguous and
every vertical shift is a free-dim offset. No on-device halo exchange.
Output is written blocked fp16 and unscrambled on the host.
"""

import sys

sys.path.insert(0, "/opt/trn_rl_repo")

import numpy as np

H = 2048
W = 2048
RAD = 5
P = 128
G = 128  # rows per partition group
NG = H // G  # 16 row groups
NCHUNK = P // NG  # 8 column chunks per superband
WB = 64  # cols per chunk
WH = WB + 12  # 76: left halo 6, right halo 5 + 1 pad (even strides)
NSB = W // (WB * NCHUNK)  # 4 superbands
XR = G + 2 * RAD  # 138 rows in x tile
N_CORES = 8

_CACHE = {}


def _build():
    import concourse.bacc as bacc
    import concourse.tile as tile
    import concourse.mybir as mybir

    f16 = mybir.dt.float16
    MAX = mybir.AluOpType.max

    nc = bacc.Bacc("TRN2", target_bir_lowering=False, debug=False, num_devices=N_CORES)
    xin = nc.dram_tensor("xin", [NSB, P, XR, WH], f16, kind="ExternalInput").ap()
    yout = nc.dram_tensor("yout", [NSB, P, G, WB], f16, kind="ExternalOutput").ap()

    with tile.TileContext(nc) as tc:
        with (
            tc.tile_pool(name="xx", bufs=2) as p_xx,
            tc.tile_pool(name="pA", bufs=1) as p_a,
            tc.tile_pool(name="pB", bufs=1) as p_b,
            tc.tile_pool(name="pC", bufs=1) as p_c,
            tc.tile_pool(name="acc", bufs=2) as p_acc,
        ):
            for b in range(NSB):
                # ---- load packed band (halos pre-baked); rows: slot i = row+5
                xx = p_xx.tile([P, XR, WH], f16, tag="xx")
                # s1[i,j] = max over x cols {j-5, j-4} at row i-4 (slot i+1 in xx)
                s1 = p_a.tile([P, G + 8, 75], f16, tag="A")
                if b == 0:
                    # split the cold-start load 4 ways so the ladder starts
                    # after the first quarter lands
                    cuts = [0, 35, 69, 104, XR]
                    for k in range(4):
                        nc.sync.dma_start(
                            xx[:, cuts[k] : cuts[k + 1], :],
                            xin[b][:, cuts[k] : cuts[k + 1], :],
                        )
                    # s1 chunk k needs xx rows [cuts[k-1]+1, cuts[k]+1) ... i.e.
                    # chunk k of s1 rows [cuts[k]-1, cuts[k+1]-1) reads xx rows
                    # [cuts[k], cuts[k+1]) plus one row from the next chunk; use
                    # rows [max(cuts[k]-1,0), min(cuts[k+1]-1, G+8)) so chunk k
                    # only reads xx rows < cuts[k+1]+... keep it simple: chunk k
                    # covers s1 rows [lo, hi) with lo=cuts[k]-1 (clamped), which
                    # reads xx rows [lo+1, hi+1) <= cuts[k+1].
                    for k in range(4):
                        lo = max(cuts[k] - 1, 0)
                        hi = min(cuts[k + 1] - 1, G + 8)
                        nc.vector.tensor_tensor(
                            s1[:, lo:hi, :],
                            xx[:, lo + 1 : hi + 1, 0:75],
                            xx[:, lo + 1 : hi + 1, 1:76],
                            op=MAX,
                        )
                else:
                    nc.sync.dma_start(xx[:, :, :], xin[b])
                    nc.vector.tensor_tensor(
                        s1[:, :, :],
                        xx[:, 1 : G + 9, 0:75],
                        xx[:, 1 : G + 9, 1:76],
                        op=MAX,
                    )
                # s2[i,j] = 4-wide, x cols j-5..j-2, rows -4..G+3 (slot i = row+4)
                s2 = p_b.tile([P, G + 8, 73], f16, tag="B")
                nc.vector.tensor_tensor(
                    s2[:, :, :], s1[:, :, 0:73], s1[:, :, 2:75], op=MAX
                )
                # h7[i,j] = 7-wide centered at col j-2, rows -4..G+3
                h7 = p_c.tile([P, G + 8, 70], f16, tag="C")
                nc.vector.tensor_tensor(
                    h7[:, :, :], s2[:, :, 0:70], s2[:, :, 3:73], op=MAX
                )
                # h9[k,j] = 9-wide centered at col j-1, rows -3..G+2 (slot k = row+3)
                h9 = p_a.tile([P, G + 6, 68], f16, tag="A")
                nc.vector.tensor_tensor(
                    h9[:, :, :], h7[:, 1 : G + 7, 0:68], h7[:, 1 : G + 7, 2:70], op=MAX
                )
                # h11 (11-wide centered, dy=0) straight into the accumulator
                acc = p_acc.tile([P, G, WB], f16, tag="acc")
                nc.vector.tensor_tensor(
                    acc[:, :, :],
                    h9[:, 3 : G + 3, 1 : 1 + WB],
                    h9[:, 3 : G + 3, 3 : 3 + WB],
                    op=MAX,
                )
                # t1[m,c] = max(h9[m], h9[m+1]) -> rows m-3..m-2, 9-wide center c
                t1 = p_b.tile([P, G + 5, WB], f16, tag="B")
                nc.vector.tensor_tensor(
                    t1[:, :, :],
                    h9[:, 0 : G + 5, 2 : 2 + WB],
                    h9[:, 1 : G + 6, 2 : 2 + WB],
                    op=MAX,
                )
                # t2[m,c] = max(t1[m], t1[m+2]) -> rows m-3..m
                t2 = p_a.tile([P, G + 3, WB], f16, tag="A")
                nc.vector.tensor_tensor(
                    t2[:, :, :], t1[:, 0 : G + 3, :], t1[:, 2 : G + 5, :], op=MAX
                )
                # acc = max(acc, t2[r], t2[r+3])       -> h9 rows r-3..r+3
                #       max(acc, h7[r-4], h7[r+4])     (h7 slot = r-+4+4, col j=c+2)
                #       max(acc, x[r-5], x[r+5])       (xx slot = r-+5+5, col c+5)
                # On the last superband, run the taps in two row-halves and
                # store the first half early to hide the output DMA tail.
                halves = [(0, G)] if b < NSB - 1 else [(0, G // 2), (G // 2, G)]
                for h0, h1 in halves:
                    a = acc[:, h0:h1, :]
                    nc.vector.tensor_tensor(a, a, t2[:, h0:h1, :], op=MAX)
                    nc.vector.tensor_tensor(a, a, t2[:, 3 + h0 : 3 + h1, :], op=MAX)
                    nc.vector.tensor_tensor(a, a, h7[:, h0:h1, 3 : 3 + WB], op=MAX)
                    nc.vector.tensor_tensor(
                        a, a, h7[:, 8 + h0 : 8 + h1, 3 : 3 + WB], op=MAX
                    )
                    nc.vector.tensor_tensor(a, a, xx[:, h0:h1, 6 : 6 + WB], op=MAX)
                    nc.vector.tensor_tensor(
                        a, a, xx[:, 10 + h0 : 10 + h1, 6 : 6 + WB], op=MAX
                    )
                    nc.scalar.dma_start(
                        yout[b][:, h0:h1, :], acc[:, h0:h1, :]
                    )

    nc.compile()
    return nc


def _get_nc():
    if "nc" not in _CACHE:
        _CACHE["nc"] = _build()
    return _CACHE["nc"]


def _pack_input(img):
    """[2048, 2048] f32 -> [NSB, 128, XR, WH] fp16 with reflect pad + halos.

    Partition p of superband s holds rows [G*g-5, G*g+G+5) and cols
    [(NCHUNK*s + c)*WB - 5, ... + WB + 5) of the original image, where
    c = p // NG, g = p % NG (indices in reflect-padded coordinates).
    """
    xpad = np.pad(img, ((RAD, RAD), (6, 6)), mode="reflect")  # [2058, 2060]
    wv = np.lib.stride_tricks.sliding_window_view(xpad, XR, axis=0)
    wv = wv[::G].transpose(0, 2, 1)  # [NG, XR, 2058]
    out = np.empty((NSB, P, XR, WH), dtype=np.float16)
    for s in range(NSB):
        for c in range(NCHUNK):
            j0 = (NCHUNK * s + c) * WB
            out[s, c * NG : (c + 1) * NG] = wv[:, :, j0 : j0 + WH]
    return out


def _unpack_output(yblk):
    """[NSB, 128, G, WB] fp16 -> [2048, 2048] f32."""
    y = np.empty((H, W), dtype=np.float32)
    for s in range(NSB):
        for c in range(NCHUNK):
            j0 = (NCHUNK * s + c) * WB
            blk = yblk[s, c * NG : (c + 1) * NG]  # [NG, G, WB]
            y[:, j0 : j0 + WB] = blk.reshape(H, WB).astype(np.float32)
    return y


def kernel(x, radius):
    from concourse.bass_utils import run_bass_kernel_spmd

    assert int(radius) == RAD
    x = np.asarray(x, dtype=np.float32)
    B, C = x.shape[0], x.shape[1]
    imgs = x.reshape(B * C, H, W)
    assert imgs.shape[0] == N_CORES

    imgs = np.where(np.isnan(imgs), np.float32(-99.0), imgs)

    nc = _get_nc()
    in_maps = [{"xin": _pack_input(imgs[c])} for c in range(N_CORES)]
    res = run_bass_kernel_spmd(nc, in_maps, core_ids=list(range(N_CORES)), trace=False)
    out = np.empty((N_CORES, H, W), dtype=np.float32)
    for c in range(N_CORES):
        out[c] = _unpack_output(res.results[c]["yout"])
    out = out.reshape(B, C, H, W)
    out = np.where(out == np.float32(-99.0), np.float32(np.nan), out)
    return out.astype(np.float32)
